# revision 20
# baseline (speedup 1.0000x reference)
"""Causal self-attention (B=4, T=2048, C=1024, 16 heads) on 8 trn2 cores.

Sharding: core c -> (batch b = c//2, head-group g = c%2 of 8 heads).
Each core computes qkv projection for its heads, causal attention, and a
partial c_proj product; the host sums the two partials per batch
(Megatron row-parallel reduce done at gather time).

Kernel layout (per core):
  - host supplies x[b].T (d-major), w slices pre-transposed, all bf16
  - qkv matmuls produce qT/kT d-major [64*2, T] per head-pair and V
    T-major [T, 8 heads, 64(+1 ones col)] for the AV matmul
  - attention computes S.T tiles [k=128 part, q<=512 free] = K Q^T,
    softmax without max-subtraction (S is O(5) so exp is safe),
    causal mask applied by accumulating a -1e9 strict-lower matrix into
    PSUM via an identity matmul (exp then underflows to 0)
  - AV: out.T[65, q] += [V|1].T @ P.T accumulated over k tiles; row 64
    is the softmax denominator (ones column trick)
  - normalize via DVE fast reciprocal + gpsimd partition_broadcast + DVE mul
  - c_proj: y.T = w_projT.T @ attT, partial over this core's channels,
    interleaved per T block with the next block's qkv/attention
"""

import math

import numpy as np
import ml_dtypes

B, T, C = 4, 2048, 1024
H = 16
D = 64
P = 128
HL = H // 2          # heads per core
NPAIR = HL // 2      # head pairs per core
KSUB = C // P        # 8 contraction subtiles for qkv
TB = 512             # T block (attention q block, qkv column block)
BF16 = ml_dtypes.bfloat16

NEG = -1.0e9
SCALE = 1.0 / math.sqrt(D)

_CACHE: dict = {}


def emit_attention(tc, io):
    """Emit the per-core kernel. io maps tensor name -> bass AP.

    Shapes (T_ may be reduced for simulation):
      xT      [C, T_]   bf16   x[b].T
      w_qk    [C, 1024] bf16   columns: [q pair0 | k pair0 | q pair1 | ...]
      w_v     [C, 512]  bf16   v weights for the 8 local heads, head-major
      w_pj    [512, C]  bf16   w_proj[:, local channels].T
      mask_lo [128,128] bf16   strict lower triangular, -1e9
      ident   [128,128] bf16   identity
      yT      [C, T_]   f32    output partial, transposed
    """
    from contextlib import ExitStack

    import concourse.mybir as mybir

    nc = tc.nc
    f32 = mybir.dt.float32
    bf = mybir.dt.bfloat16
    EXP = mybir.ActivationFunctionType.Exp

    xT, w_qk, w_v, w_pj = io["xT"], io["w_qk"], io["w_v"], io["w_pj"]
    mask_lo, ident, yT = io["mask_lo"], io["ident"], io["yT"]

    T_ = xT.shape[1]
    NTB = T_ // TB       # number of 512-wide T blocks (= q blocks)
    NKT = T_ // P        # number of 128-row k tiles

    xT_r = xT.rearrange("(ko p) t -> p ko t", p=P)      # [128, 8, T]
    wqk_r = w_qk.rearrange("(ko p) n -> p ko n", p=P)   # [128, 8, 1024]
    wv_r = w_v.rearrange("(ko p) n -> p ko n", p=P)     # [128, 8, 512]
    wpj_r = w_pj.rearrange("(ko p) n -> p ko n", p=P)   # [128, 4, 1024]
    yT_r = yT.rearrange("(yt p) t -> p yt t", p=P)      # [128, 8, T]

    marks = []

    def mark(name):
        marks.append((name, nc.next_id()))

    with ExitStack() as ctx:
        const = ctx.enter_context(tc.tile_pool(name="const", bufs=1))
        persist = ctx.enter_context(tc.tile_pool(name="persist", bufs=1))
        work = ctx.enter_context(tc.tile_pool(name="work", bufs=3))
        psum = ctx.enter_context(tc.tile_pool(name="psum", bufs=3, space="PSUM"))

        # ---- constants ----
        mark("setup")
        wqk_sb = const.tile([P, KSUB, 2 * HL * D], bf, tag="wqk")
        nc.sync.dma_start(wqk_sb, wqk_r)
        wv_sb = const.tile([P, KSUB, HL * D], bf, tag="wv")
        nc.sync.dma_start(wv_sb, wv_r)
        wpj_sb = const.tile([P, HL * D // P, C], bf, tag="wpj")
        nc.sync.dma_start(wpj_sb, wpj_r)
        mask_sb = const.tile([P, P], bf, tag="mask")
        nc.sync.dma_start(mask_sb, mask_lo)
        ident_sb = const.tile([P, P], bf, tag="ident")
        nc.sync.dma_start(ident_sb, ident)

        # ---- persistent intermediates ----
        qT_sb = [persist.tile([P, T_], bf, tag=f"qT{p}", name=f"qT{p}")
                 for p in range(NPAIR)]
        kT_sb = [persist.tile([P, T_], bf, tag=f"kT{p}", name=f"kT{p}")
                 for p in range(NPAIR)]
        # V in T-major laid out [1 | 0*63 | v*64] per head so that the AV
        # output's denominator row lands on PSUM partition 0 (where
        # reciprocal_approx_fast can read it) and the v rows span PSUM
        # partitions 64..127 (a >32-partition DVE read must start at 0 or 64).
        # M=128 costs nothing: matmul time is driven by the free dim only.
        VA = 128
        v_aug = persist.tile([P, NKT, HL, VA], bf, tag="vaug")
        nc.gpsimd.memset(v_aug[:, :, :, 0:64], 0.0)
        nc.gpsimd.memset(v_aug[:, :, :, 0], 1.0)
        attT_sb = persist.tile([P, NPAIR, T_], bf, tag="attT")

        emit_filler_ref = [lambda n=1: None]

        def attn_block(p, qb):
            """Attention for head pair p, query block qb (q in [qb*512, qb*512+512))."""
            av = [psum.tile([P, TB], f32, tag="av", bufs=2, name=f"av_{p}_{qb}_{h}")
                  for h in range(2)]
            n_full = 4 * qb

            # full k tiles, processed in pairs sharing one 2-bank psum tile.
            # Per-head chains (ST -> exp -> AV) so the PE runs head h1's STs
            # while ACT computes head h0's exp.
            for i in range(0, n_full, 2):
                for h in range(2):
                    d0, d1 = 64 * h, 64 * h + 64
                    hg = 2 * p + h
                    st = psum.tile([P, 2 * TB], f32, tag="st", bufs=2, name=f"st_{p}_{qb}_{i}_{h}")
                    mark("stfull")
                    for j in range(2):
                        kt = i + j
                        nc.tensor.matmul(
                            st[:, j * TB:(j + 1) * TB],
                            lhsT=kT_sb[p][d0:d1, kt * P:(kt + 1) * P],
                            rhs=qT_sb[p][d0:d1, qb * TB:(qb + 1) * TB],
                            start=True, stop=True,
                        )
                    pt = work.tile([P, 2 * TB], bf, tag="pt", bufs=4, name=f"pt_{p}_{qb}_{i}_{h}")
                    mark("exp")
                    nc.scalar.activation(pt, st, EXP, scale=SCALE)
                    mark("av")
                    for j in range(2):
                        kt = i + j
                        nc.tensor.matmul(
                            av[h][0:VA, :],
                            lhsT=v_aug[:, kt, hg, :],
                            rhs=pt[:, j * TB:(j + 1) * TB],
                            start=(kt == 0), stop=False,
                            skip_group_check=True,
                        )
                    emit_filler_ref[0](1)

            # diagonal k tiles with causal mask. The mask matmul accumulates
            # into the same psum region as the ST matmul; keeping the two
            # heads' chains separate lets ACT/PE ping-pong across heads.
            for j in range(4):
                kt = n_full + j
                ncols = TB - j * P
                for h in range(2):
                    d0, d1 = 64 * h, 64 * h + 64
                    hg = 2 * p + h
                    std = psum.tile([P, 2 * TB], f32, tag="st", bufs=2, name=f"std_{p}_{qb}_{j}_{h}")
                    mark("stdiag")
                    nc.tensor.matmul(
                        std[:, 0:ncols],
                        lhsT=kT_sb[p][d0:d1, kt * P:(kt + 1) * P],
                        rhs=qT_sb[p][d0:d1, qb * TB + j * P:(qb + 1) * TB],
                        start=True, stop=False,
                        skip_group_check=True,
                    )
                    # add -1e9 above the causal diagonal of the 128x128 subtile
                    nc.tensor.matmul(
                        std[:, 0:P],
                        lhsT=ident_sb,
                        rhs=mask_sb,
                        start=False, stop=True,
                        skip_group_check=True,
                    )
                    ptd = work.tile([P, TB], bf, tag="ptd", bufs=4, name=f"ptd_{p}_{qb}_{j}_{h}")
                    mark("expd")
                    nc.scalar.activation(ptd[:, 0:ncols], std[:, 0:ncols], EXP, scale=SCALE)
                    mark("avd")
                    nc.tensor.matmul(
                        av[h][0:VA, j * P:TB],
                        lhsT=v_aug[:, kt, hg, :],
                        rhs=ptd[:, 0:ncols],
                        start=(kt == 0), stop=(j == 3),
                        skip_group_check=True,
                    )
                    if h == 1:
                        emit_filler_ref[0](1)

            # normalize by the denominator (AV row 0, on PSUM partition 0 where
            # reciprocal_approx_fast can read it directly) and store to attT
            # normalize by the denominator (AV row 0). Copy the whole AV psum
            # tile to SBUF first so the psum slot frees after one DVE op; the
            # denominator lands on partition 0 of the copy, where
            # reciprocal_approx_fast can read it (it breaks at base_part != 0).
            for h in range(2):
                mark("norm")
                ao = work.tile([P, TB], f32, tag="ao", bufs=2, name=f"ao_{p}_{qb}_{h}")
                nc.vector.tensor_copy(out=ao, in_=av[h])
                rcp = work.tile([1, TB], f32, tag="rcp", bufs=2, name=f"rcp_{p}_{qb}_{h}")
                nc.vector.reciprocal_approx_fast(out=rcp, in_=ao[0:1, :])
                bc = work.tile([P, TB], f32, tag="bc", bufs=2, name=f"bc_{p}_{qb}_{h}")
                nc.gpsimd.partition_broadcast(bc, rcp)
                nc.vector.tensor_mul(
                    out=attT_sb[64 * h:64 * h + 64, p, qb * TB:(qb + 1) * TB],
                    in0=ao[64:64 + D, :],
                    in1=bc[64:64 + D, :],
                )

        # ---- filler work: qkv / proj psum groups fed into attention stalls ----
        # The PE stream is in-order, so exp-wait bubbles inside the attention
        # stretch can only be filled by emitting independent matmul groups
        # between attention units. qkv of the NEXT T block and c_proj of the
        # PREVIOUS T block are both dependency-free at that point.
        from collections import deque
        filler_q = deque()

        def emit_filler(n=1):
            for _ in range(n):
                if filler_q:
                    filler_q.popleft()()

        emit_filler_ref[0] = emit_filler

        def qkv_qk_group(xt, tb, wt):
            def go():
                ps = psum.tile([P, TB], f32, tag="fill", bufs=2, name=f"qk_ps_{tb}_{wt}")
                mark("qkvqk")
                for ks in range(KSUB):
                    nc.tensor.matmul(
                        ps,
                        lhsT=wqk_sb[:, ks, wt * P:(wt + 1) * P],
                        rhs=xt[:, ks, :],
                        start=(ks == 0), stop=(ks == KSUB - 1),
                    )
                pr, isk = divmod(wt, 2)
                dst = (kT_sb if isk else qT_sb)[pr][:, tb * TB:(tb + 1) * TB]
                mark("qkcopy")
                nc.vector.tensor_copy(out=dst, in_=ps)
            return go

        def qkv_v_group(xt, tb, tt):
            def go():
                psv = psum.tile([P, HL * D], f32, tag="fill", bufs=2, name=f"v_ps_{tb}_{tt}")
                mark("qkvv")
                for ks in range(KSUB):
                    nc.tensor.matmul(
                        psv,
                        lhsT=xt[:, ks, tt * P:(tt + 1) * P],
                        rhs=wv_sb[:, ks, :],
                        start=(ks == 0), stop=(ks == KSUB - 1),
                    )
                kt_idx = tb * (TB // P) + tt
                mark("vcopy")
                nc.vector.tensor_copy(
                    out=v_aug[:, kt_idx, :, 64:64 + D],
                    in_=psv.rearrange("p (h d) -> p h d", h=HL),
                )
            return go

        def proj_group(tb, yrt):
            def go():
                pj = psum.tile([P, TB], f32, tag="fill", bufs=2, name=f"pj_{yrt}_{tb}")
                mark("proj")
                for ks in range(NPAIR):
                    nc.tensor.matmul(
                        pj,
                        lhsT=wpj_sb[:, ks, yrt * P:(yrt + 1) * P],
                        rhs=attT_sb[:, ks, tb * TB:(tb + 1) * TB],
                        start=(ks == 0), stop=(ks == NPAIR - 1),
                    )
                mark("yout")
                yo = work.tile([P, TB], f32, tag="yo", name=f"yo_{yrt}_{tb}")
                nc.vector.tensor_copy(out=yo, in_=pj)
                nc.sync.dma_start(yT_r[:, yrt, tb * TB:(tb + 1) * TB], yo)
            return go

        def emit_xt_dma(tb):
            mark("xtdma")
            xt = work.tile([P, KSUB, TB], bf, tag="xt", bufs=2, name=f"xt_{tb}")
            nc.sync.dma_start(xt, xT_r[:, :, tb * TB:(tb + 1) * TB])
            return xt

        # ---- main loop ----
        # qkv(0) runs up front; afterwards qkv(tb+1) and proj(tb-1) are
        # emitted as fillers inside attention(tb).
        xt0 = emit_xt_dma(0)
        for g in [qkv_qk_group(xt0, 0, wt) for wt in range(2 * NPAIR)] + \
                 [qkv_v_group(xt0, 0, tt) for tt in range(TB // P)]:
            g()

        for tb in range(NTB):
            if tb + 1 < NTB:
                xt = emit_xt_dma(tb + 1)
                for wt in range(2 * NPAIR):
                    filler_q.append(qkv_qk_group(xt, tb + 1, wt))
                for tt in range(TB // P):
                    filler_q.append(qkv_v_group(xt, tb + 1, tt))
            if tb - 1 >= 0:
                for yrt in range(C // P):
                    filler_q.append(proj_group(tb - 1, yrt))

            for p in range(NPAIR):
                attn_block(p, qb=tb)
                emit_filler(2)

            while filler_q:
                emit_filler(1)

        for yrt in range(C // P):
            proj_group(NTB - 1, yrt)()

        mark("end")

    return marks


def _build(T_=T):
    if T_ in _CACHE:
        return _CACHE[T_]
    import concourse.bacc as bacc
    import concourse.mybir as mybir
    import concourse.tile as tile

    nc = bacc.Bacc("TRN2", debug=False, num_devices=8)
    bf = mybir.dt.bfloat16
    f32 = mybir.dt.float32
    io = {
        "xT": nc.dram_tensor("xT", [C, T_], bf, kind="ExternalInput").ap(),
        "w_qk": nc.dram_tensor("w_qk", [C, 2 * HL * D], bf, kind="ExternalInput").ap(),
        "w_v": nc.dram_tensor("w_v", [C, HL * D], bf, kind="ExternalInput").ap(),
        "w_pj": nc.dram_tensor("w_pj", [HL * D, C], bf, kind="ExternalInput").ap(),
        "mask_lo": nc.dram_tensor("mask_lo", [P, P], bf, kind="ExternalInput").ap(),
        "ident": nc.dram_tensor("ident", [P, P], bf, kind="ExternalInput").ap(),
        "yT": nc.dram_tensor("yT", [C, T_], f32, kind="ExternalOutput").ap(),
    }
    with tile.TileContext(nc) as tc:
        marks = emit_attention(tc, io)
    try:
        import json
        with open("/tmp/phase_marks.json", "w") as f:
            json.dump(marks, f)
    except Exception:
        pass
    nc.compile()
    _CACHE[T_] = nc
    return nc


def make_core_inputs(x, w_attn, w_proj, core, T_=T):
    """Host-side sharding for one core: (batch, head-group) slice + relayout."""
    b, g = divmod(core, 2)
    gs = slice(g * HL * D, (g + 1) * HL * D)
    q, k, v = w_attn[0:C], w_attn[C:2 * C], w_attn[2 * C:3 * C]
    qg, kg, vg = q[gs], k[gs], v[gs]          # [512, C] each
    blocks = []
    for p in range(NPAIR):
        blocks.append(qg[p * P:(p + 1) * P])
        blocks.append(kg[p * P:(p + 1) * P])
    wqk = np.concatenate(blocks, axis=0).T    # [C, 1024]
    return {
        "xT": np.ascontiguousarray(x[b, :T_].T).astype(BF16),
        "w_qk": np.ascontiguousarray(wqk).astype(BF16),
        "w_v": np.ascontiguousarray(vg.T).astype(BF16),
        "w_pj": np.ascontiguousarray(w_proj[:, gs].T).astype(BF16),
        "mask_lo": np.tril(np.full((P, P), NEG, np.float32), -1).astype(BF16),
        "ident": np.eye(P, dtype=np.float32).astype(BF16),
    }


def kernel(x, w_attn, w_proj):
    x = np.asarray(x, dtype=np.float32)
    w_attn = np.asarray(w_attn, dtype=np.float32)
    w_proj = np.asarray(w_proj, dtype=np.float32)

    from concourse.bass_utils import run_bass_kernel_spmd

    nc = _build()
    in_maps = [make_core_inputs(x, w_attn, w_proj, c) for c in range(8)]
    res = run_bass_kernel_spmd(nc, in_maps, core_ids=list(range(8)))

    y = np.empty((B, T, C), dtype=np.float32)
    for b in range(B):
        yT = res.results[2 * b]["yT"] + res.results[2 * b + 1]["yT"]
        y[b] = yT.T
    return y
